# revision 28
# baseline (speedup 1.0000x reference)
"""Trainium2 Bass kernel for DeformBottleneckBlock (DCNv2 bottleneck).

Sharding: 8 cores = (batch b in 0..3) x (H-half in 0..1); each core computes
output rows [lo, lo+50) of one image. Fully data-parallel, no collectives.

Per-core pipeline (v3 — DRAM token gather, token-major combine, slim xbar):
  A) conv1 1x1 (bf16, bn1 folded, bias via indicator channel) -> out1
     channel-major bf16 [128, 2, 58, 128]; 116 HWDGE xbar transposes build
     tmS2[x, y, 2ct, 128ch]; 4 shifted DMA writes build a DRAM 4-corner
     token table tm_d[y0*128+x0, 1024el] (2KB tokens = rows y0,y0+1 x cols
     x0,x0+1 x 256ch).
  B) offset conv 3x3 -> om [27,5120].  Field pipeline on [90,512] tiles
     (row = k*10 + chunk, col = position%512): corner weight maps W00..11
     and int16 token indices (clamped).  Indices bounce through DRAM with a
     slot permutation sigma(i) = (i%16)*32 + i//16 per 512-chunk so the
     16-wrapped gather layout reads back as contiguous 64B runs.  Weight
     maps are PE-transposed (16 tiny transposes) into wT[128slot, c, g, 90]
     for per-partition use in stage C.
  C) per (k, 512-chunk): one non-transpose dma_gather from DRAM (2KB
     descriptors) -> G[128, 4, 1024] token-major; combine corners in
     token-major with per-partition scalars (2 DVE tensor_scalar + 2 ACT
     scale-muls + 3 DVE adds per group) -> s_tok[128, 4, 256]; 4 xbar
     transposes (only the combined samples: 4x less than corner data) ->
     sT[128ch, g, ct, 128]; conv2 matmuls consume sT through an
     inverse-permutation access pattern so PSUM columns are
     position-ordered; bn2 bias + relu -> out2 chunk.
  D) conv3 1x1 (bf16) + residual add (bf16 x) + bn3 bias + relu -> out
     (bf16, upcast on host), interleaved per chunk with stage C.
"""

import numpy as np
import ml_dtypes

B, CIN, H, W = 4, 1024, 100, 100
CB, COUT, KOFF = 256, 1024, 27

PAD = 4
RSTRIP = 58
XP = 128                       # padded x pitch
NPOS = 5120
NCHUNK = 512
NCHUNKS = NPOS // NCHUNK       # 10
NVALID = 5000
NTOK = RSTRIP * 128            # 7424 tokens (y0 in 0..57, x0 in 0..127)
TELEM = 1024                   # elements per token (bf16) = 2KB
NR = 90                        # field rows = 9 taps x 10 chunks
NF = 512                       # field cols = positions per chunk


def _build_program():
    import concourse.bacc as bacc
    import concourse.mybir as mybir
    from concourse.tile import TileContext
    from concourse.bass import ts
    from concourse.masks import make_identity

    dt = mybir.dt
    AF = mybir.ActivationFunctionType
    ALU = mybir.AluOpType
    f32, bf16, i16, i32 = dt.float32, dt.bfloat16, dt.int16, dt.int32

    nc = bacc.Bacc("TRN2", target_bir_lowering=False, num_swdge_queues=4)

    xs_d = nc.dram_tensor("xs", [8, 128, RSTRIP * W], bf16, kind="ExternalInput")
    ind_d = nc.dram_tensor("ind", [1, RSTRIP * W], bf16, kind="ExternalInput")
    w1T_d = nc.dram_tensor("w1T", [8, 128, CB], bf16, kind="ExternalInput")
    w1b_d = nc.dram_tensor("w1b", [1, CB], bf16, kind="ExternalInput")
    woffT_d = nc.dram_tensor("woffT", [9, 2, 128, KOFF], bf16, kind="ExternalInput")
    boff_d = nc.dram_tensor("boff", [KOFF, 1], f32, kind="ExternalInput")
    w2T_d = nc.dram_tensor("w2T", [9, 2, 128, CB], bf16, kind="ExternalInput")
    b2_d = nc.dram_tensor("b2", [128, 2], f32, kind="ExternalInput")
    w3T_d = nc.dram_tensor("w3T", [2, 128, COUT], bf16, kind="ExternalInput")
    b3_d = nc.dram_tensor("b3", [128, 8], f32, kind="ExternalInput")
    basey_d = nc.dram_tensor("basey", [NR, NF], f32, kind="ExternalInput")
    basex_d = nc.dram_tensor("basex", [NR, NF], f32, kind="ExternalInput")
    kia_d = nc.dram_tensor("kia", [NR, 1], f32, kind="ExternalInput")
    kja_d = nc.dram_tensor("kja", [NR, 1], f32, kind="ExternalInput")
    vb_d = nc.dram_tensor("vb", [NR, 4], f32, kind="ExternalInput")
    tm_d = nc.dram_tensor("tm_tokens", [NTOK, TELEM], bf16)
    out_d = nc.dram_tensor("out", [8, 128, NVALID], bf16, kind="ExternalOutput")

    with TileContext(nc) as tc:
        with tc.tile_pool(name="persist", bufs=1) as pp, \
             tc.tile_pool(name="io", bufs=2) as iop:

            w00 = pp.tile([NR, NF], bf16)
            w01 = pp.tile([NR, NF], bf16)
            w10 = pp.tile([NR, NF], bf16)
            w11 = pp.tile([NR, NF], bf16)
            wT = pp.tile([128, 4, 4, NR], f32)       # [slot, corner, g, row]
            idx_top = pp.tile([128, 9 * 320], i16)
            b2 = pp.tile([128, 2], f32)
            nc.sync.dma_start(out=b2, in_=b2_d[:, :])
            b3 = pp.tile([128, 8], f32)
            nc.sync.dma_start(out=b3, in_=b3_d[:, :])

            with tc.tile_pool(name="omscope", bufs=1) as omp:
                om = omp.tile([KOFF, NPOS], f32)
                tmS2 = omp.tile([128, RSTRIP + 1, 2, 128], bf16)
                tmS2b = omp.tile([128, RSTRIP + 1, 2, 128], bf16)

                # ======== Stage A: conv1, token table build, offset conv ====
                with tc.tile_pool(name="stageab", bufs=1) as ap, \
                     tc.tile_pool(name="xck", bufs=3) as xp, \
                     tc.tile_pool(name="psA", bufs=2, space="PSUM") as psA:

                    nc.vector.memset(tmS2[:, RSTRIP, :, :], 0)
                    nc.vector.memset(tmS2b[:, RSTRIP, :, :], 0)
                    out1_cm = ap.tile([128, 2, RSTRIP * XP], bf16)
                    nc.vector.memset(out1_cm[:, :, :], 0)
                    out1_cm2 = ap.tile([128, 2, RSTRIP * XP], bf16)
                    nc.vector.memset(out1_cm2[:, :, :], 0)
                    w1T = ap.tile([128, 8, CB], bf16)
                    for kt in range(8):
                        nc.sync.dma_start(out=w1T[:, kt, :], in_=w1T_d[kt, :, :])
                    w1b = ap.tile([1, CB], bf16)
                    nc.sync.dma_start(out=w1b, in_=w1b_d[:, :])
                    woffT = ap.tile([128, 9, 2, KOFF], bf16)
                    for tap in range(9):
                        for ct in range(2):
                            nc.sync.dma_start(out=woffT[:, tap, ct, :],
                                              in_=woffT_d[tap, ct, :, :])
                    boff = ap.tile([KOFF, 1], f32)
                    nc.sync.dma_start(out=boff, in_=boff_d[:, :])

                    cmv = out1_cm.rearrange("p c (r w) -> p c r w", w=XP)
                    cmv2 = out1_cm2.rearrange("p c (r w) -> p c r w", w=XP)

                    def om_chunk(rc):
                        r0 = rc * 5
                        npos = 5 * W
                        ps = psA.tile([KOFF, 5 * W], f32, tag="omps")
                        first = True
                        for tap in range(9):
                            ti, tj = divmod(tap, 3)
                            rhs = cmv[:, :, r0 + 3 + ti:r0 + 3 + ti + 5,
                                      PAD + tj - 1:PAD + tj - 1 + W]
                            for ct in range(2):
                                nc.tensor.matmul(
                                    ps.rearrange("p (r w) -> p r w", w=W),
                                    woffT[:, tap, ct, :], rhs[:, ct],
                                    start=first, stop=(tap == 8 and ct == 1))
                                first = False
                        nc.scalar.activation(om[:, rc * npos:(rc + 1) * npos], ps,
                                             AF.Identity, bias=boff[:, :])

                    chunks = [(4 * i, 4) for i in range(14)] + [(56, 2)]
                    for (r0, nrows) in chunks:
                        npos = nrows * W
                        xt = xp.tile([128, 8, 4 * W], bf16, tag="xchunk")
                        for kt in range(8):
                            nc.sync.dma_start(out=xt[:, kt, :npos],
                                              in_=xs_d[kt, :, r0 * W:r0 * W + npos])
                        indt = xp.tile([1, 4 * W], bf16, tag="indchunk")
                        nc.sync.dma_start(out=indt[:, :npos],
                                          in_=ind_d[:, r0 * W:r0 * W + npos])
                        for mt in range(2):
                            ps = psA.tile([128, 4 * W], f32, tag="convps")
                            for kt in range(8):
                                nc.tensor.matmul(ps[:, :npos], w1T[:, kt, ts(mt, 128)],
                                                 xt[:, kt, :npos],
                                                 start=(kt == 0), stop=False)
                            nc.tensor.matmul(ps[:, :npos], w1b[:, ts(mt, 128)],
                                             indt[:, :npos], start=False, stop=True)
                            psv = ps[:, :npos].rearrange("p (r w) -> p r w", w=W)
                            nc.scalar.activation(
                                cmv[:, mt, r0:r0 + nrows, PAD:PAD + W], psv, AF.Relu)
                            nc.scalar.activation(
                                cmv2[:, mt, r0:r0 + nrows, PAD - 1:PAD - 1 + W],
                                psv, AF.Relu)
                        # batched xbar transposes for the finished rows
                        for ct in range(2):
                            nc.sync.dma_start(
                                out=tmS2[:, r0:r0 + nrows, ct, :],
                                in_=cmv[:, ct, r0:r0 + nrows, :].rearrange(
                                    "p r w -> p (r w)"),
                                transpose=True)
                            nc.sync.dma_start(
                                out=tmS2b[:, r0:r0 + nrows, ct, :],
                                in_=cmv2[:, ct, r0:r0 + nrows, :].rearrange(
                                    "p r w -> p (r w)"),
                                transpose=True)
                    for rc in range(10):
                        om_chunk(rc)

                # ======== Stage B2: field pipeline on [90, 512] tiles ========
                with tc.tile_pool(name="fieldsc", bufs=1) as fc, \
                     tc.tile_pool(name="psB", bufs=2, space="PSUM") as psB:
                    _tc_n = [0]

                    def T(tag, d=f32):
                        _tc_n[0] += 1
                        return fc.tile([NR, NF], d, tag=tag,
                                       name=f"fld_{tag}_{_tc_n[0]}")

                    dyp = T("pA")
                    dxp = T("pB")
                    mrp = T("pC")
                    basey = T("pD")
                    basex = T("pE")
                    for k in range(9):
                        nc.sync.dma_start(
                            out=dyp[10 * k:10 * k + 10, :],
                            in_=om[2 * k:2 * k + 1, :].rearrange(
                                "q (c n) -> q c n", n=NF))
                        nc.sync.dma_start(
                            out=dxp[10 * k:10 * k + 10, :],
                            in_=om[2 * k + 1:2 * k + 2, :].rearrange(
                                "q (c n) -> q c n", n=NF))
                        nc.sync.dma_start(
                            out=mrp[10 * k:10 * k + 10, :],
                            in_=om[18 + k:19 + k, :].rearrange(
                                "q (c n) -> q c n", n=NF))
                    nc.sync.dma_start(out=basey, in_=basey_d[:, :])
                    nc.sync.dma_start(out=basex, in_=basex_d[:, :])
                    # token table writes: tm_d[(y0,x0)] = [v00, v10, v01, v11]
                    # (issued after the field DMAs so B2 isn't queued behind them)
                    tmdv = tm_d.rearrange("(y x) (c e) -> x y c e", x=128, c=4)
                    nc.sync.dma_start(out=tmdv[:, :, 0, :],
                                      in_=tmS2[:, 0:RSTRIP, :, :])
                    nc.sync.dma_start(out=tmdv[:, :, 1, :],
                                      in_=tmS2[:, 1:RSTRIP + 1, :, :])
                    nc.sync.dma_start(out=tmdv[:, :, 2, :],
                                      in_=tmS2b[:, 0:RSTRIP, :, :])
                    nc.sync.dma_start(out=tmdv[:, :, 3, :],
                                      in_=tmS2b[:, 1:RSTRIP + 1, :, :])
                    kia = fc.tile([NR, 1], f32)
                    nc.sync.dma_start(out=kia, in_=kia_d[:, :])
                    kja = fc.tile([NR, 1], f32)
                    nc.sync.dma_start(out=kja, in_=kja_d[:, :])
                    vb = fc.tile([NR, 4], f32)
                    nc.sync.dma_start(out=vb, in_=vb_d[:, :])

                    ayy = T("pF")
                    nc.vector.tensor_add(ayy, dyp, basey)          # pA,pD free
                    nc.scalar.activation(ayy, ayy, AF.Identity, bias=kia[:, :])
                    ayi = T("pA", i32)
                    nc.vector.tensor_copy(ayi, ayy)                # floor(yy)+8
                    ayf = T("pD")
                    nc.vector.tensor_copy(ayf, ayi)                # pA free
                    wyh = T("pG")                                  # wy - 0.5
                    nc.vector.tensor_sub(wyh, ayy, ayf)            # pF free
                    msig = T("pF")
                    nc.scalar.activation(msig, mrp, AF.Sigmoid)    # pC free
                    bxx = T("pC")
                    nc.vector.tensor_add(bxx, dxp, basex)          # pB,pE free
                    nc.scalar.activation(bxx, bxx, AF.Identity, bias=kja[:, :])
                    bxi = T("pB", i32)
                    nc.vector.tensor_copy(bxi, bxx)
                    bxf = T("pE")
                    nc.vector.tensor_copy(bxf, bxi)                # pB free
                    wxh = T("pH")
                    nc.vector.tensor_sub(wxh, bxx, bxf)            # pC free

                    # token index from floors (ayf, bxf live):
                    # idx = (ayf-8)*128 + (bxf-8), clamped to [0, NTOK-1]
                    idxpf = T("pB")
                    nc.vector.tensor_scalar(idxpf, ayf, 128.0, -1032.0,
                                            ALU.mult, ALU.add)
                    nc.vector.tensor_add(idxpf, idxpf, bxf)
                    nc.vector.tensor_scalar(idxpf, idxpf, 0.0, float(NTOK - 1),
                                            ALU.max, ALU.min)
                    # 16-wrap relayout on-chip: PE-transpose [90,16] column
                    # blocks of the (still-f32) index field into idx_top rows
                    # 0..15, then replicate to the other 7 core groups.
                    identf = fc.tile([NR, NR], f32)
                    make_identity(nc, identf)
                    idxv = idx_top[0:16, :].rearrange("p (k c s) -> p k c s",
                                                      k=9, c=10)
                    for s in range(32):
                        psi = psB.tile([16, NR], f32, tag="idxtp")
                        nc.tensor.transpose(psi, idxpf[:, 16 * s:16 * s + 16],
                                            identf)
                        nc.scalar.copy(idxv[:, :, :, s],
                                       psi.rearrange("p (k c) -> p k c", c=10))
                    for g in range(1, 8):
                        nc.sync.dma_start(out=idx_top[16 * g:16 * g + 16, :],
                                          in_=idx_top[0:16, :])

                    # validity + weight maps
                    def cmp_range(dst, src, lo_ap, hi_ap, tmp):
                        nc.vector.tensor_scalar(tmp, src, lo_ap, None, ALU.is_ge)
                        nc.vector.tensor_scalar(dst, src, hi_ap, None, ALU.is_le)
                        nc.vector.tensor_mul(dst, dst, tmp)

                    tmp = T("pA")
                    vy0 = T("pB")
                    cmp_range(vy0, ayf, vb[:, 0:1], vb[:, 1:2], tmp)
                    vy1 = T("pC")
                    cmp_range(vy1, ayf, vb[:, 2:3], vb[:, 3:4], tmp)   # pD free
                    atop = T("pD")
                    nc.vector.tensor_scalar(atop, wyh, -1.0, 0.5, ALU.mult, ALU.add)
                    nc.vector.tensor_mul(atop, atop, msig)
                    nc.vector.tensor_mul(atop, atop, vy0)              # pB free
                    abot = T("pB")
                    nc.vector.tensor_scalar(abot, wyh, 0.5, None, ALU.add)
                    nc.vector.tensor_mul(abot, abot, msig)
                    nc.vector.tensor_mul(abot, abot, vy1)   # pG, pF, pC free
                    vx0 = T("pC")
                    cmp_range(vx0, bxf, 12.0, 111.0, tmp)
                    vx1 = T("pF")
                    cmp_range(vx1, bxf, 11.0, 110.0, tmp)              # pE free
                    c0 = T("pE")
                    nc.vector.tensor_scalar(c0, wxh, -1.0, 0.5, ALU.mult, ALU.add)
                    nc.vector.tensor_mul(c0, c0, vx0)                  # pC free
                    c1 = T("pC")
                    nc.vector.tensor_scalar(c1, wxh, 0.5, None, ALU.add)
                    nc.vector.tensor_mul(c1, c1, vx1)                  # pH, pF free
                    nc.vector.tensor_mul(w00, atop, c0)
                    nc.vector.tensor_mul(w01, atop, c1)
                    nc.vector.tensor_mul(w10, abot, c0)
                    nc.vector.tensor_mul(w11, abot, c1)

                    # distribute weight maps to slot-major wT via PE transpose:
                    # wT[p, corner, g, r] = wmap[r, g*128+p]
                    identw = fc.tile([NR, NR], bf16)
                    make_identity(nc, identw)
                    # corner order matches the token block layout: 00, 10, 01, 11
                    for ci, wmap in enumerate((w00, w10, w01, w11)):
                        for g in range(4):
                            pst = psB.tile([128, NR], bf16, tag="wtp")
                            nc.tensor.transpose(pst, wmap[:, 128 * g:128 * (g + 1)],
                                                identw)
                            nc.scalar.copy(wT[:, ci, g, :], pst)

            # ======== Stage C + D (interleaved per chunk) ========
            with tc.tile_pool(name="stagecd", bufs=1) as cp, \
                 tc.tile_pool(name="gath", bufs=6) as gp, \
                 tc.tile_pool(name="psC", bufs=2, space="PSUM") as psC:

                out2 = cp.tile([128, 2, NPOS], bf16)
                w2T = cp.tile([128, 9, 2, CB], bf16)
                for k in range(9):
                    for ct in range(2):
                        nc.sync.dma_start(out=w2T[:, k, ct, :], in_=w2T_d[k, ct, :, :])
                w3T = cp.tile([128, 2, COUT], bf16)
                for ct in range(2):
                    nc.sync.dma_start(out=w3T[:, ct, :], in_=w3T_d[ct, :, :])
                identr = cp.tile([128, 128], bf16)
                make_identity(nc, identr)

                for ch in range(NCHUNKS):
                    accs = [psC.tile([128, NCHUNK], f32, tag=f"dacc{mt}",
                                     name=f"dacc_{ch}_{mt}")
                            for mt in range(2)]
                    for k in range(9):
                        r = k * 10 + ch
                        G = gp.tile([128, 4, TELEM], bf16, tag="gtok")
                        isl = idx_top[:, k * 320 + ch * 32:k * 320 + (ch + 1) * 32]
                        nc.gpsimd.dma_gather(
                            out_ap=G[:, :, :], in_ap=tm_d[:, :], idxs_ap=isl,
                            num_idxs=NCHUNK, num_idxs_reg=NCHUNK,
                            elem_size=TELEM, queue_num=k % 4)
                        Gv = G.rearrange("p g (c t e) -> p g c t e", c=4, t=2)
                        s_tok = gp.tile([128, 2, 4, 128], bf16, tag="stok")
                        m1a = gp.tile([128, 4, 2, 128], bf16, tag="m1")
                        m2a = gp.tile([128, 2, 4, 128], bf16, tag="m2")
                        m3a = gp.tile([128, 4, 2, 128], bf16, tag="m3")
                        for g in range(4):
                            nc.vector.tensor_scalar(s_tok[:, :, g, :],
                                                    Gv[:, g, 0, :, :],
                                                    wT[:, 0, g, r:r + 1], None,
                                                    ALU.mult)
                            nc.scalar.activation(m1a[:, g, :, :], Gv[:, g, 1, :, :],
                                                 AF.Identity,
                                                 scale=wT[:, 1, g, r:r + 1])
                            nc.vector.tensor_scalar(m2a[:, :, g, :],
                                                    Gv[:, g, 2, :, :],
                                                    wT[:, 2, g, r:r + 1], None,
                                                    ALU.mult)
                            nc.scalar.activation(m3a[:, g, :, :], Gv[:, g, 3, :, :],
                                                 AF.Identity,
                                                 scale=wT[:, 3, g, r:r + 1])
                        m1v = m1a.rearrange("p g c e -> p c g e")
                        m3v = m3a.rearrange("p g c e -> p c g e")
                        nc.vector.tensor_add(s_tok, s_tok, m1v)
                        nc.vector.tensor_add(m2a, m2a, m3v)
                        nc.vector.tensor_add(s_tok, s_tok, m2a)
                        sT = gp.tile([128, 2, 4, 128], bf16, tag="sT")
                        nc.sync.dma_start(out=sT.rearrange("p c g e -> p (c g) e"),
                                          in_=s_tok.rearrange("p c g e -> p (c g e)"),
                                          transpose=True)
                        for mt in range(2):
                            for ct in range(2):
                                nc.tensor.matmul(accs[mt], w2T[:, k, ct, ts(mt, 128)],
                                                 sT[:, ct, :, :],
                                                 start=(k == 0 and ct == 0),
                                                 stop=(k == 8 and ct == 1))
                    for mt in range(2):
                        nc.scalar.activation(out2[:, mt, ch * NCHUNK:(ch + 1) * NCHUNK],
                                             accs[mt], AF.Relu, bias=b2[:, mt:mt + 1])

                    # ---- Stage D for this chunk ----
                    n0 = ch * NCHUNK
                    nn = min(NCHUNK, NVALID - n0)
                    if nn <= 0:
                        continue
                    for mt in range(8):
                        xr = iop.tile([128, NCHUNK], bf16, tag="xres")
                        nc.sync.dma_start(out=xr[:, :nn],
                                          in_=xs_d[mt, :, PAD * W + n0:PAD * W + n0 + nn])
                        ps = psC.tile([128, NCHUNK], f32, tag="c3ps", bufs=2,
                                      name=f"c3ps_{ch}_{mt}")
                        for ct in range(2):
                            nc.tensor.matmul(ps[:, :nn], w3T[:, ct, ts(mt, 128)],
                                             out2[:, ct, n0:n0 + nn],
                                             start=(ct == 0), stop=False)
                        nc.tensor.matmul(ps[:, :nn], identr, xr[:, :nn],
                                         start=False, stop=True)
                        o = iop.tile([128, NCHUNK], bf16, tag="obuf")
                        nc.scalar.activation(o[:, :nn], ps[:, :nn], AF.Relu,
                                             bias=b3[:, mt:mt + 1])
                        nc.sync.dma_start(out=out_d[mt, :, n0:n0 + nn], in_=o[:, :nn])

    nc.finalize()
    return nc


_NC_CACHE = None


def _get_nc():
    global _NC_CACHE
    if _NC_CACHE is None:
        _NC_CACHE = _build_program()
    return _NC_CACHE


def _prep_inputs(x, w1, s1, b1, w_off, b_off, w2, s2, b2, w3, s3, b3):
    bf16 = ml_dtypes.bfloat16
    f32 = np.float32
    x = np.asarray(x, f32)
    w1f = np.asarray(w1, f32) * np.asarray(s1, f32)[:, None]
    w1T = np.ascontiguousarray(w1f.T.reshape(8, 128, CB)).astype(bf16)
    w1b = np.ascontiguousarray(np.asarray(b1, f32)[None, :]).astype(bf16)
    woffT = np.zeros((9, 2, 128, KOFF), bf16)
    w_off = np.asarray(w_off, f32)
    for tap in range(9):
        ti, tj = divmod(tap, 3)
        wt = w_off[:, :, ti, tj]
        for ct in range(2):
            woffT[tap, ct] = wt[:, ct * 128:(ct + 1) * 128].T.astype(bf16)
    boff = np.asarray(b_off, f32)[:, None]
    w2f = np.asarray(w2, f32) * np.asarray(s2, f32)[:, None, None, None]
    w2T = np.zeros((9, 2, 128, CB), bf16)
    for k in range(9):
        ki, kj = divmod(k, 3)
        wk = w2f[:, :, ki, kj]
        for ct in range(2):
            w2T[k, ct] = wk[:, ct * 128:(ct + 1) * 128].T.astype(bf16)
    b2t = np.ascontiguousarray(np.asarray(b2, f32).reshape(2, 128).T)
    w3f = np.asarray(w3, f32) * np.asarray(s3, f32)[:, None]
    w3T = np.zeros((2, 128, COUT), bf16)
    for ct in range(2):
        w3T[ct] = w3f[:, ct * 128:(ct + 1) * 128].T.astype(bf16)
    b3t = np.ascontiguousarray(np.asarray(b3, f32).reshape(8, 128).T)

    j = np.arange(NPOS)
    y_loc = np.where(j < NVALID, PAD + j // W, 20).astype(f32)
    x_pad = np.where(j < NVALID, PAD + j % W, 50).astype(f32)
    basey = np.zeros((NR, NF), f32)
    basex = np.zeros((NR, NF), f32)
    kia = np.zeros((NR, 1), f32)
    kja = np.zeros((NR, 1), f32)
    for k in range(9):
        ki, kj = divmod(k, 3)
        for c in range(10):
            basey[10 * k + c] = y_loc[c * NF:(c + 1) * NF]
            basex[10 * k + c] = x_pad[c * NF:(c + 1) * NF]
            kia[10 * k + c] = ki - 1 + 7.5
            kja[10 * k + c] = kj - 1 + 7.5

    shared = dict(w1T=w1T, w1b=w1b, woffT=np.asarray(woffT), boff=boff,
                  w2T=np.asarray(w2T), b2=b2t, w3T=np.asarray(w3T), b3=b3t,
                  basey=basey, basex=basex, kia=kia, kja=kja)

    in_maps = []
    for core in range(8):
        b, half = core // 2, core % 2
        lo = half * 50
        xs = np.zeros((CIN, RSTRIP, W), f32)
        vlo = max(0, lo - PAD)
        vhi = min(H - 1, lo + 49 + PAD)
        loc0 = vlo - (lo - PAD)
        nrows = vhi - vlo + 1
        xs[:, loc0:loc0 + nrows, :] = x[b, :, vlo:vhi + 1, :]
        indv = np.zeros((RSTRIP, W), f32)
        indv[loc0:loc0 + nrows, :] = 1.0
        vbm = np.zeros((NR, 4), f32)
        vbm[:, 0] = loc0 + 8
        vbm[:, 1] = loc0 + nrows - 1 + 8
        vbm[:, 2] = loc0 + 8 - 1
        vbm[:, 3] = loc0 + nrows - 1 + 8 - 1
        in_maps.append(dict(shared,
                            xs=np.ascontiguousarray(
                                xs.reshape(8, 128, RSTRIP * W)).astype(bf16),
                            ind=indv.reshape(1, -1).astype(bf16), vb=vbm))
    return in_maps


def kernel(**inputs):
    from concourse.bass_utils import run_bass_kernel_spmd
    nc = _get_nc()
    in_maps = _prep_inputs(**inputs)
    res = run_bass_kernel_spmd(nc, in_maps, core_ids=list(range(8)))
    out = np.zeros((B, COUT, H, W), np.float32)
    for core in range(8):
        b, half = core // 2, core % 2
        lo = half * 50
        o = res.results[core]["out"].reshape(COUT, 50, W).astype(np.float32)
        out[b, :, lo:lo + 50, :] = o
    return out
